# revision 1
# baseline (speedup 1.0000x reference)
"""Trainium2 Bass kernel for nn_DDNWithResidualLoss.

Contract: kernel(**inputs) takes the FULL unsharded inputs (numpy arrays,
keyed as in reference.setup_inputs()) and returns the FULL output (the two
scalar losses). The batch dim B=8 is sharded 1 image per NeuronCore across
8 cores; the box list shards with its image; per-core partial weighted sums
are combined on the host (the cross-device psum is 16 floats).

Key observation: the per-pixel target bin t takes at most 17 distinct
values per image (16 boxes + background), so the channel gather
x[t[p], p] is a one-hot matmul. Per 128-pixel chunk, one PE matmul of the
exp'd logits chunk ([81, 128], stationary) against [ones | H] ([81, 18])
yields out[p, :] = [ sum_c e[c,p] | e[c_j, p] (17) ] — the softmax
denominator and the 17 exp-candidates at once. A 17-way select keyed on t
(tensor_scalar is_equal masks + copy_predicated) picks the target-bin
value per pixel.

The residual tensor is only ever read at those same <=17 channels; the
host gathers the 17 candidate rows and lays them out pixel-major
([128, 240*17]) while sharding, so the device reads 2.1 MB instead of
9.95 MB and selects r_t with the same masks. Box rasterization + LID
depth binning involve only the tiny box inputs (640 floats); they are
replicated bit-exactly on the host in float32 and shipped as small
per-pixel auxiliary planes (target bin, residual target, fg/bg weight).

Schedule: logits stream in 8 blocks of 3840 pixels; exp on ScalarE in
half-blocks; PE matmuls per 128-px chunk into PSUM groups; DVE drains the
groups and runs the selects per block so everything overlaps the DMA
stream; the loss epilogue runs in two column-halves to pipeline the
ACT/DVE dependency chain.
"""

import numpy as np

# ---------------- problem constants (hardcoded per contract) ----------------
B, D, H, W = 8, 80, 96, 320
C = D + 1              # 81 channels
HW = H * W             # 30720 pixels
P = 128                # SBUF partitions per chunk
NCH = HW // P          # 240 chunks of 128 pixels
NCAND = 17             # max distinct target bins per image (16 boxes + bg)
XJ = 1 + NCAND         # [s | e-cands] = 18 columns per chunk
ALPHA, GAMMA = 0.25, 2.0
FG_W, BG_W = 13.0, 1.0
DEPTH_MIN, DEPTH_MAX = 0.001, 60.0
EPS = 1e-8
N_CORES = 8

f32 = np.float32


# ---------------- host-side reference-exact target computation ----------------
def _host_targets(gt_boxes2d, num_gt_per_img, gt_center_depth):
    """Bit-exact float32 replication of the reference's rasterization+binning.

    Returns per-pixel planes (B, H, W): depth bin target (int32),
    residual target (f32), balancer weight (f32).
    """
    gt_boxes2d = np.asarray(gt_boxes2d, f32)
    gt_center_depth = np.asarray(gt_center_depth, f32)
    num_gt = np.asarray(num_gt_per_img, np.int64)

    u1 = np.floor(gt_boxes2d[:, 0]).astype(np.int32)
    v1 = np.floor(gt_boxes2d[:, 1]).astype(np.int32)
    u2 = np.ceil(gt_boxes2d[:, 2]).astype(np.int32)
    v2 = np.ceil(gt_boxes2d[:, 3]).astype(np.int32)
    ntot = gt_boxes2d.shape[0]

    # jnp.repeat(..., total_repeat_length=ntot): truncate, or pad with the
    # final value (matches jax semantics for the padded tail).
    rep = np.repeat(np.arange(B), np.clip(num_gt, 0, None))
    if len(rep) >= ntot:
        rep = rep[:ntot]
    else:
        pad_val = rep[-1] if len(rep) else 0
        rep = np.concatenate([rep, np.full(ntot - len(rep), pad_val, rep.dtype)])

    dm = np.full((B, H, W), DEPTH_MAX, f32)
    fg = np.zeros((B, H, W), bool)
    for i in range(ntot):
        b = int(rep[i])
        ys = slice(max(int(v1[i]), 0), max(int(v2[i]), 0))
        xs = slice(max(int(u1[i]), 0), max(int(u2[i]), 0))
        dm[b, ys, xs] = np.minimum(dm[b, ys, xs], gt_center_depth[i])
        fg[b, ys, xs] = True

    num_bins = D
    bin_size = f32(2.0 * (DEPTH_MAX - DEPTH_MIN) / (num_bins * (1 + num_bins)))
    with np.errstate(invalid="ignore"):
        idx = f32(-0.5) + f32(0.5) * np.sqrt(
            f32(1.0) + f32(8.0) * (dm - f32(DEPTH_MIN)) / bin_size, dtype=f32
        )
        bad = (idx < 0) | (idx > num_bins) | ~np.isfinite(idx)
        tgt = np.where(bad, num_bins, np.floor(np.where(bad, 0, idx))).astype(np.int32)

    bi = np.arange(num_bins, dtype=f32)
    bin_value = (bi + f32(0.5)) ** 2 * bin_size / f32(2.0) - bin_size / f32(8.0) + f32(DEPTH_MIN)
    bin_values = np.concatenate([bin_value, np.array([DEPTH_MAX], f32)])

    res_tgt = (dm - bin_values[tgt]).astype(f32)
    wgt = np.where(fg, f32(FG_W), f32(BG_W))
    return tgt, res_tgt, wgt


def _pmajor(plane):
    """(H*W,) raster vector -> [128, 240] tile, pixel i=128k+p at [p, k]."""
    return np.ascontiguousarray(plane.reshape(NCH, P).T)


# ---------------- device program ----------------
_PROGRAM = None

BLK = 3840               # pixels per staged DMA block (15 KB/partition)
NBLK = HW // BLK         # 8 blocks
CPB = BLK // P           # 30 chunks per block
GRP = 15                 # chunks per PSUM group (15*18=270 <= 512), 2 per block
EPI_SPLIT = 2            # epilogue column-halves (pipeline the ACT/DVE chain)


def _build_program(loop_iters=None):
    """Build the SPMD program. loop_iters (benchmark only): wrap the body in
    an on-device For loop so one NEFF executes the kernel body N times,
    letting wall-clock measurements amortize launch/transfer overhead."""
    import concourse.tile as tile
    from concourse import bacc, mybir
    from contextlib import ExitStack, nullcontext

    dt = mybir.dt
    Alu = mybir.AluOpType
    Act = mybir.ActivationFunctionType

    nc = bacc.Bacc("TRN2", target_bir_lowering=False, debug=False)

    x_d = nc.declare_dram_parameter("x", [C, HW], dt.float32, isOutput=False)
    rc_d = nc.declare_dram_parameter("rcand", [P, NCH * NCAND], dt.float32,
                                     isOutput=False)
    rh_d = nc.declare_dram_parameter("rh", [C, XJ], dt.float32, isOutput=False)
    cb_d = nc.declare_dram_parameter("cb", [P, NCAND], dt.float32, isOutput=False)
    t_d = nc.declare_dram_parameter("tf", [P, NCH], dt.float32, isOutput=False)
    rt_d = nc.declare_dram_parameter("rt", [P, NCH], dt.float32, isOutput=False)
    w_d = nc.declare_dram_parameter("w", [P, NCH], dt.float32, isOutput=False)
    out_d = nc.declare_dram_parameter("out", [P, 2], dt.float32, isOutput=True)

    GPB = CPB // GRP     # psum groups per block

    with tile.TileContext(nc) as tc, ExitStack() as ctx:
        const_p = ctx.enter_context(tc.tile_pool(name="const", bufs=1))
        stage_p = ctx.enter_context(tc.tile_pool(name="stage", bufs=2))
        psum_p = ctx.enter_context(tc.tile_pool(name="psum", bufs=4, space="PSUM"))
        small_p = ctx.enter_context(tc.tile_pool(name="small", bufs=1))
        blk_p = ctx.enter_context(tc.tile_pool(name="blk", bufs=2))

        rh = const_p.tile([C, XJ], dt.float32)
        nc.sync.dma_start(out=rh[:], in_=rh_d[:])
        cb = const_p.tile([P, NCAND], dt.float32)
        nc.sync.dma_start(out=cb[:], in_=cb_d[:])
        eps_t = const_p.tile([P, 1], dt.float32)
        nc.gpsimd.memset(eps_t[:], EPS)
        t_t = small_p.tile([P, NCH], dt.float32)
        nc.sync.dma_start(out=t_t[:], in_=t_d[:])
        rt_t = small_p.tile([P, NCH], dt.float32)
        nc.sync.dma_start(out=rt_t[:], in_=rt_d[:])
        w_t = small_p.tile([P, NCH], dt.float32)
        nc.sync.dma_start(out=w_t[:], in_=w_d[:])
        rcand = small_p.tile([P, NCH * NCAND], dt.float32)
        nc.sync.dma_start(out=rcand[:], in_=rc_d[:])

        loop_cm = (tc.For_i(0, loop_iters, 1, hint_engines=(nc.tensor.engine,))
                   if loop_iters else nullcontext())
        ctx.enter_context(loop_cm)

        # all 16 selection masks depend only on t: build once, reuse per block
        masks = []
        for j in range(1, NCAND):
            mask = small_p.tile([P, NCH], dt.uint8, tag=f"mask{j}")
            nc.vector.tensor_scalar(mask[:], t_t[:], cb[:, j:j + 1], None,
                                    op0=Alu.is_equal)
            masks.append(mask)

        part = small_p.tile([P, 2], dt.float32)
        s_t = small_p.tile([P, NCH], dt.float32)
        et_t = small_p.tile([P, NCH], dt.float32)
        rp_t = small_p.tile([P, NCH], dt.float32)

        # residual select runs once up front (independent of the x stream)
        rcv = rcand[:].rearrange("p (k j) -> p k j", j=NCAND)
        nc.vector.tensor_copy(rp_t[:], rcv[:, :, 0])
        for j in range(1, NCAND):
            nc.vector.copy_predicated(rp_t[:], masks[j - 1][:], rcv[:, :, j])

        for blk in range(NBLK):
            ks = slice(blk * CPB, (blk + 1) * CPB)   # this block's chunk cols
            xs = stage_p.tile([C, BLK], dt.float32, tag="xs")
            nc.sync.dma_start(out=xs[:], in_=x_d[:, blk * BLK:(blk + 1) * BLK])
            es = stage_p.tile([C, BLK], dt.float32, tag="es")
            nc.scalar.activation(es[:, 0:BLK // 2], xs[:, 0:BLK // 2], Act.Exp)
            nc.scalar.activation(es[:, BLK // 2:BLK], xs[:, BLK // 2:BLK], Act.Exp)

            xc = blk_p.tile([P, CPB * XJ], dt.float32, tag="xc")
            for g in range(GPB):
                pg = psum_p.tile([P, GRP * XJ], dt.float32, tag="pg", space="PSUM")
                for j in range(GRP):
                    kl = g * GRP + j
                    nc.tensor.matmul(pg[:, j * XJ:(j + 1) * XJ],
                                     es[:, kl * P:(kl + 1) * P], rh[:],
                                     start=True, stop=True)
                nc.vector.tensor_copy(
                    xc[:, g * GRP * XJ:(g + 1) * GRP * XJ], pg[:])

            # ---- select at target bin (this block's 30 chunk-columns) ----
            xcv = xc[:].rearrange("p (k j) -> p k j", j=XJ)
            nc.vector.tensor_copy(s_t[:, ks], xcv[:, :, 0])
            nc.vector.tensor_copy(et_t[:, ks], xcv[:, :, 1])
            for j in range(1, NCAND):
                nc.vector.copy_predicated(et_t[:, ks], masks[j - 1][:, ks],
                                          xcv[:, :, 1 + j])

        # ---- loss epilogue, in column-halves to pipeline ACT/DVE ----
        mapacc = small_p.tile([P, EPI_SPLIT], dt.float32)
        resacc = small_p.tile([P, EPI_SPLIT], dt.float32)
        EW = NCH // EPI_SPLIT
        for h in range(EPI_SPLIT):
            hs = slice(h * EW, (h + 1) * EW)
            ln_et = blk_p.tile([P, EW], dt.float32, tag="ln_et")
            nc.scalar.activation(ln_et[:], et_t[:, hs], Act.Ln)
            ln_s = blk_p.tile([P, EW], dt.float32, tag="ln_s")
            nc.scalar.activation(ln_s[:], s_t[:, hs], Act.Ln)
            q = blk_p.tile([P, EW], dt.float32, tag="q")
            nc.vector.tensor_sub(q[:], ln_et[:], ln_s[:])
            praw = blk_p.tile([P, EW], dt.float32, tag="praw")
            nc.scalar.activation(praw[:], q[:], Act.Exp)          # p = e_t / s
            lnp = blk_p.tile([P, EW], dt.float32, tag="lnp")
            nc.scalar.activation(lnp[:], praw[:], Act.Ln, bias=eps_t[:])
            u = blk_p.tile([P, EW], dt.float32, tag="u")
            nc.vector.tensor_scalar(u[:], praw[:], -1.0, 1.0,
                                    op0=Alu.mult, op1=Alu.add)
            focal = blk_p.tile([P, EW], dt.float32, tag="focal")
            nc.scalar.activation(focal[:], u[:], Act.Square, scale=0.5)
            m1 = blk_p.tile([P, EW], dt.float32, tag="m1")
            nc.vector.tensor_mul(m1[:], focal[:], lnp[:])
            m1w = blk_p.tile([P, EW], dt.float32, tag="m1w")
            nc.vector.tensor_mul(m1w[:], m1[:], w_t[:, hs])
            nc.vector.tensor_reduce(mapacc[:, h:h + 1], m1w[:],
                                    axis=mybir.AxisListType.X, op=Alu.add)
            dres = blk_p.tile([P, EW], dt.float32, tag="dres")
            nc.vector.tensor_sub(dres[:], rp_t[:, hs], rt_t[:, hs])
            ares = blk_p.tile([P, EW], dt.float32, tag="ares")
            nc.scalar.activation(ares[:], dres[:], Act.Abs)
            m2 = blk_p.tile([P, EW], dt.float32, tag="m2")
            nc.vector.tensor_mul(m2[:], ares[:], focal[:])
            m2w = blk_p.tile([P, EW], dt.float32, tag="m2w")
            nc.vector.tensor_mul(m2w[:], m2[:], w_t[:, hs])
            nc.vector.tensor_reduce(resacc[:, h:h + 1], m2w[:],
                                    axis=mybir.AxisListType.X, op=Alu.add)

        acc0 = small_p.tile([P, 1], dt.float32)
        nc.vector.tensor_reduce(acc0[:], mapacc[:], axis=mybir.AxisListType.X,
                                op=Alu.add)
        nc.vector.tensor_scalar(part[:, 0:1], acc0[:], -1.0, None, op0=Alu.mult)
        nc.vector.tensor_reduce(part[:, 1:2], resacc[:],
                                axis=mybir.AxisListType.X, op=Alu.add)
        nc.sync.dma_start(out=out_d[:], in_=part[:])

    nc.compile()
    return nc


def _get_program():
    global _PROGRAM
    if _PROGRAM is None:
        _PROGRAM = _build_program()
    return _PROGRAM


LAST_RESULTS = None  # populated with the BassKernelResults of the last run


def _build_in_maps(depth_logits, depth_residuals, tgt, res_tgt, wgt):
    """depth_logits/depth_residuals: (B, C, HW); tgt/res_tgt/wgt: (B, ...)"""
    in_maps = []
    for b in range(N_CORES):
        tgt_b = tgt[b].reshape(HW)
        c_list = np.unique(tgt_b)
        assert len(c_list) <= NCAND, f"more than {NCAND} distinct bins"
        c_list = np.concatenate(
            [c_list, np.full(NCAND - len(c_list), c_list[0], c_list.dtype)])
        rh = np.zeros((C, XJ), f32)
        rh[:, 0] = 1.0
        rh[c_list, np.arange(1, XJ)] = 1.0
        cb = np.tile(c_list.astype(f32), (P, 1))
        # candidate residual rows, pixel-major: rcand[p, k*17+j] = r[c_j, 128k+p]
        r17 = depth_residuals[b].reshape(C, HW)[c_list]          # [17, HW]
        rcand = np.ascontiguousarray(
            r17.reshape(NCAND, NCH, P).transpose(2, 1, 0).reshape(P, NCH * NCAND))
        in_maps.append({
            "x": depth_logits[b].reshape(C, HW),
            "rcand": rcand,
            "rh": rh,
            "cb": np.ascontiguousarray(cb),
            "tf": _pmajor(tgt_b.astype(f32)),
            "rt": _pmajor(res_tgt[b].reshape(HW)),
            "w": _pmajor(wgt[b].reshape(HW)),
        })
    return in_maps


def kernel(depth_logits, depth_residuals, gt_boxes2d, num_gt_per_img, gt_center_depth):
    global LAST_RESULTS
    from concourse.bass_utils import run_bass_kernel_spmd

    depth_logits = np.ascontiguousarray(np.asarray(depth_logits, f32))
    depth_residuals = np.ascontiguousarray(np.asarray(depth_residuals, f32))

    tgt, res_tgt, wgt = _host_targets(gt_boxes2d, num_gt_per_img, gt_center_depth)
    in_maps = _build_in_maps(depth_logits.reshape(B, C, HW),
                             depth_residuals.reshape(B, C, HW),
                             tgt, res_tgt, wgt)

    nc = _get_program()
    res = run_bass_kernel_spmd(nc, in_maps, list(range(N_CORES)))
    LAST_RESULTS = res

    acc = np.zeros(2, np.float64)
    for b in range(N_CORES):
        acc += np.asarray(res.results[b]["out"], np.float64).sum(axis=0)
    num_pixels = float(B * H * W)
    map_loss = f32(acc[0] / num_pixels)
    res_loss = f32(acc[1] / num_pixels)
    return map_loss, res_loss



# revision 7
# speedup vs baseline: 1.0496x; 1.0496x over previous
"""Trainium2 Bass kernel for nn_DDNWithResidualLoss.

Contract: kernel(**inputs) takes the FULL unsharded inputs (numpy arrays,
keyed as in reference.setup_inputs()) and returns the FULL output (the two
scalar losses). The batch dim B=8 is sharded 1 image per NeuronCore across
8 cores; the box list shards with its image; per-core partial weighted sums
are combined on the host (the cross-device psum is 16 floats).

Architecture (v2, pixel-major, no matmuls):
  The loss is a weighted SUM over pixels, so the host may permute pixels
  freely while sharding. Logits ship PIXEL-MAJOR fp16: x[p, k*81+c] holds
  pixel (p,k)'s 81 channels contiguously. ScalarE streams exp over the
  whole tensor (1 elem/lane/cycle); the softmax denominator s is a
  per-pixel sum over the 81 contiguous channels, computed as a 5-level
  pairwise tensor_tensor ADD tree on DVE in fp16 (2x throughput mode).

  The per-pixel target bin takes <=17 distinct values per image (16 boxes
  + background). The host sorts pixels so each 16-partition x column cell
  is target-homogeneous, which makes the per-pixel channel select a GpSimd
  indirect_copy (per-16-partition-group shared u16 indices, 4-byte chunks:
  gather fp16 PAIRS, keep element 0). The same gather picks the candidate
  residual r_t from a host-gathered 17-row candidate table. Box
  rasterization + LID binning touch only the tiny box inputs and are
  replicated bit-exactly on the host; per-pixel aux (residual target,
  fg/bg weight) ship as fp16 planes. The focal/log epilogue runs on
  DVE/ScalarE over [128, 244] tiles with the final weighted sums fused
  into tensor_tensor_reduce accumulators.
"""

import numpy as np

# ---------------- problem constants (hardcoded per contract) ----------------
B, D, H, W = 8, 80, 96, 320
C = D + 1              # 81 channels
HW = H * W             # 30720 pixels
P = 128                # SBUF partitions
NCAND = 17             # max distinct target bins per image (16 boxes + bg)
NGRP = P // 16         # 8 gpsimd index groups
K = 244                # pixel columns: 8 groups * 244 cells >= 1937 needed
KB = 61                # columns per stream block
NBLK = K // KB         # 4 blocks
NEL = K * C            # 19764 elements per partition (x row)
XROW = NEL + 2         # +2 pad for the d=2 gather tail
RROW = K * NCAND       # 4148 (residual candidate row)
RROWP = RROW + 2       # 4150, even, +2 pad for gather tail
SIDX = 16              # wrapped index columns (16*16=256 >= K)
KH0 = 128              # es-gather half 0 columns (byte offsets < 32KB each)
KH1 = K - KH0          # 116
SIDXH = 8              # wrapped index columns per half (8*16=128 >= 128,116)
ALPHA = 0.25
FG_W, BG_W = 13.0, 1.0
DEPTH_MIN, DEPTH_MAX = 0.001, 60.0
N_CORES = 8

f32 = np.float32
f16 = np.float16


# ---------------- host-side reference-exact target computation ----------------
def _host_targets(gt_boxes2d, num_gt_per_img, gt_center_depth):
    """Bit-exact float32 replication of the reference's rasterization+binning.

    Returns per-pixel planes (B, H, W): depth bin target (int32),
    residual target (f32), balancer weight (f32).
    """
    gt_boxes2d = np.asarray(gt_boxes2d, f32)
    gt_center_depth = np.asarray(gt_center_depth, f32)
    num_gt = np.asarray(num_gt_per_img, np.int64)

    u1 = np.floor(gt_boxes2d[:, 0]).astype(np.int32)
    v1 = np.floor(gt_boxes2d[:, 1]).astype(np.int32)
    u2 = np.ceil(gt_boxes2d[:, 2]).astype(np.int32)
    v2 = np.ceil(gt_boxes2d[:, 3]).astype(np.int32)
    ntot = gt_boxes2d.shape[0]

    # jnp.repeat(..., total_repeat_length=ntot): truncate, or pad with the
    # final value (matches jax semantics for the padded tail).
    rep = np.repeat(np.arange(B), np.clip(num_gt, 0, None))
    if len(rep) >= ntot:
        rep = rep[:ntot]
    else:
        pad_val = rep[-1] if len(rep) else 0
        rep = np.concatenate([rep, np.full(ntot - len(rep), pad_val, rep.dtype)])

    dm = np.full((B, H, W), DEPTH_MAX, f32)
    fg = np.zeros((B, H, W), bool)
    for i in range(ntot):
        b = int(rep[i])
        ys = slice(max(int(v1[i]), 0), max(int(v2[i]), 0))
        xs = slice(max(int(u1[i]), 0), max(int(u2[i]), 0))
        dm[b, ys, xs] = np.minimum(dm[b, ys, xs], gt_center_depth[i])
        fg[b, ys, xs] = True

    num_bins = D
    bin_size = f32(2.0 * (DEPTH_MAX - DEPTH_MIN) / (num_bins * (1 + num_bins)))
    with np.errstate(invalid="ignore"):
        idx = f32(-0.5) + f32(0.5) * np.sqrt(
            f32(1.0) + f32(8.0) * (dm - f32(DEPTH_MIN)) / bin_size, dtype=f32
        )
        bad = (idx < 0) | (idx > num_bins) | ~np.isfinite(idx)
        tgt = np.where(bad, num_bins, np.floor(np.where(bad, 0, idx))).astype(np.int32)

    bi = np.arange(num_bins, dtype=f32)
    bin_value = (bi + f32(0.5)) ** 2 * bin_size / f32(2.0) - bin_size / f32(8.0) + f32(DEPTH_MIN)
    bin_values = np.concatenate([bin_value, np.array([DEPTH_MAX], f32)])

    res_tgt = (dm - bin_values[tgt]).astype(f32)
    wgt = np.where(fg, f32(FG_W), f32(BG_W))
    return tgt, res_tgt, wgt


# ---------------- device program ----------------
_PROGRAM = None


def _build_program():
    import concourse.tile as tile
    from concourse import bacc, mybir
    from contextlib import ExitStack

    dt = mybir.dt
    Alu = mybir.AluOpType
    Act = mybir.ActivationFunctionType

    nc = bacc.Bacc("TRN2", target_bir_lowering=False, debug=False)

    x_d = nc.declare_dram_parameter("x", [P, NEL], dt.float16, isOutput=False)
    rc_d = nc.declare_dram_parameter("rc", [P, RROWP], dt.float16, isOutput=False)
    ie0_d = nc.declare_dram_parameter("ie0", [P, SIDXH], dt.uint16, isOutput=False)
    ie1_d = nc.declare_dram_parameter("ie1", [P, SIDXH], dt.uint16, isOutput=False)
    ir_d = nc.declare_dram_parameter("ir", [P, SIDX], dt.uint16, isOutput=False)
    rt_d = nc.declare_dram_parameter("rt", [P, K], dt.float16, isOutput=False)
    w_d = nc.declare_dram_parameter("w", [P, K], dt.float16, isOutput=False)
    out_d = nc.declare_dram_parameter("out", [P, 2], dt.float32, isOutput=True)

    with tile.TileContext(nc) as tc, ExitStack() as ctx:
        main_p = ctx.enter_context(tc.tile_pool(name="main", bufs=1))
        stage_p = ctx.enter_context(tc.tile_pool(name="stage", bufs=2))

        rc_t = main_p.tile([P, RROWP], dt.float16)
        nc.sync.dma_start(out=rc_t[:], in_=rc_d[:])
        ie0_t = main_p.tile([P, SIDXH], dt.uint16)
        nc.sync.dma_start(out=ie0_t[:], in_=ie0_d[:])
        ie1_t = main_p.tile([P, SIDXH], dt.uint16)
        nc.sync.dma_start(out=ie1_t[:], in_=ie1_d[:])
        ir_t = main_p.tile([P, SIDX], dt.uint16)
        nc.sync.dma_start(out=ir_t[:], in_=ir_d[:])
        rt_t = main_p.tile([P, K], dt.float16)
        nc.sync.dma_start(out=rt_t[:], in_=rt_d[:])
        w_t = main_p.tile([P, K], dt.float16)
        nc.sync.dma_start(out=w_t[:], in_=w_d[:])

        es = main_p.tile([P, XROW], dt.float16)
        nc.gpsimd.memset(es[:, NEL:XROW], 0.0)
        t1 = main_p.tile([P, K, 40], dt.float16)
        t2 = main_p.tile([P, K, 20], dt.float16)
        t3 = main_p.tile([P, K, 10], dt.float16)
        t4 = main_p.tile([P, K, 5], dt.float16)
        t5 = main_p.tile([P, K, 2], dt.float16)
        ua = main_p.tile([P, K], dt.float32)
        ub = main_p.tile([P, K], dt.float32)
        s_t = main_p.tile([P, K], dt.float32)

        for b in range(NBLK):
            cs = slice(b * KB * C, (b + 1) * KB * C)
            ks = slice(b * KB, (b + 1) * KB)
            xs = stage_p.tile([P, KB * C], dt.float16, tag="xs")
            nc.sync.dma_start(out=xs[:], in_=x_d[:, cs])
            nc.scalar.activation(es[:, cs], xs[:], Act.Exp)

            ev = es[:, cs].rearrange("p (k c) -> p k c", c=C)
            t1s = t1[:, ks, :]
            t2s = t2[:, ks, :]
            t3s = t3[:, ks, :]
            t4s = t4[:, ks, :]
            t5s = t5[:, ks, :]
            with nc.allow_low_precision("fp16 softmax-denominator tree"):
                nc.vector.tensor_tensor(t1s, ev[:, :, 0:40], ev[:, :, 40:80],
                                        op=Alu.add)
                nc.vector.tensor_tensor(t2s, t1s[:, :, 0:20], t1s[:, :, 20:40],
                                        op=Alu.add)
                nc.vector.tensor_tensor(t3s, t2s[:, :, 0:10], t2s[:, :, 10:20],
                                        op=Alu.add)
                nc.vector.tensor_tensor(t4s, t3s[:, :, 0:5], t3s[:, :, 5:10],
                                        op=Alu.add)
                nc.vector.tensor_tensor(t5s, t4s[:, :, 0:2], t4s[:, :, 2:4],
                                        op=Alu.add)
            nc.vector.tensor_tensor(ua[:, ks], t5s[:, :, 0], t5s[:, :, 1],
                                    op=Alu.add)
            nc.vector.tensor_tensor(ub[:, ks], t4s[:, :, 4], ev[:, :, 80],
                                    op=Alu.add)
            nc.vector.tensor_tensor(s_t[:, ks], ua[:, ks], ub[:, ks],
                                    op=Alu.add)

        # ---- per-pixel target-bin selects (gpsimd gather, fp16 pairs) ----
        # es-gather byte offsets must stay < 32KB (gpsimd ucode limit):
        # gather in two column-halves against rebased data views
        et2 = main_p.tile([P, K, 2], dt.float16)
        ev0 = es[:, 0:KH0 * C + 2].rearrange("p (q two) -> p q two", two=2)
        nc.gpsimd.indirect_copy(et2[:, 0:KH0, :], ev0, ie0_t[:], True)
        ev1 = es[:, KH0 * C:XROW].rearrange("p (q two) -> p q two", two=2)
        nc.gpsimd.indirect_copy(et2[:, KH0:K, :], ev1, ie1_t[:], True)
        rs2 = main_p.tile([P, K, 2], dt.float16)
        nc.gpsimd.indirect_copy(
            rs2[:], rc_t[:].rearrange("p (q two) -> p q two", two=2), ir_t[:], True)

        # ---- focal/log epilogue; alpha and /num_pixels folded on host ----
        rec = main_p.tile([P, K], dt.float32)
        nc.vector.reciprocal(rec[:], s_t[:])
        pt = main_p.tile([P, K], dt.float32)
        nc.vector.tensor_tensor(pt[:], et2[:, :, 0], rec[:], op=Alu.mult)
        lnp = main_p.tile([P, K], dt.float32)
        nc.scalar.activation(lnp[:], pt[:], Act.Ln)
        u = main_p.tile([P, K], dt.float32)
        nc.vector.tensor_scalar(u[:], pt[:], -1.0, 1.0, op0=Alu.mult,
                                op1=Alu.add)
        focal = main_p.tile([P, K], dt.float32)
        nc.vector.tensor_tensor(focal[:], u[:], u[:], op=Alu.mult)
        lw = main_p.tile([P, K], dt.float32)
        nc.vector.tensor_tensor(lw[:], lnp[:], w_t[:], op=Alu.mult)
        part = main_p.tile([P, 2], dt.float32)
        scr = main_p.tile([P, K], dt.float32)
        nc.vector.tensor_tensor(scr[:], focal[:], lw[:], op=Alu.mult)
        nc.vector.tensor_reduce(part[:, 0:1], scr[:],
                                axis=mybir.AxisListType.X, op=Alu.add)
        dres = main_p.tile([P, K], dt.float32)
        nc.vector.tensor_tensor(dres[:], rs2[:, :, 0], rt_t[:],
                                op=Alu.subtract)
        ndres = main_p.tile([P, K], dt.float32)
        nc.vector.tensor_scalar(ndres[:], dres[:], -1.0, None, op0=Alu.mult)
        ares = main_p.tile([P, K], dt.float32)
        nc.vector.tensor_tensor(ares[:], dres[:], ndres[:], op=Alu.max)
        fw = main_p.tile([P, K], dt.float32)
        nc.vector.tensor_tensor(fw[:], focal[:], w_t[:], op=Alu.mult)
        scr2 = main_p.tile([P, K], dt.float32)
        nc.vector.tensor_tensor(scr2[:], ares[:], fw[:], op=Alu.mult)
        nc.vector.tensor_reduce(part[:, 1:2], scr2[:],
                                axis=mybir.AxisListType.X, op=Alu.add)
        nc.sync.dma_start(out=out_d[:], in_=part[:])

    nc.compile()
    return nc


def _get_program():
    global _PROGRAM
    if _PROGRAM is None:
        _PROGRAM = _build_program()
    return _PROGRAM


LAST_RESULTS = None  # populated with the BassKernelResults of the last run


def _wrap_idx(lin_idx, scols):
    """(NGRP, n) linear gather indices -> [P, scols] u16 wrapped layout."""
    n = lin_idx.shape[1]
    out = np.zeros((P, scols), np.uint16)
    k = np.arange(n)
    for g in range(NGRP):
        out[16 * g + (k % 16), k // 16] = lin_idx[g]
    return out


def _build_in_maps(depth_logits, depth_residuals, tgt, res_tgt, wgt):
    """depth_logits/depth_residuals: (B, C, HW); tgt/res_tgt/wgt: (B, ...)."""
    in_maps = []
    ncells_grid = NGRP * K
    for b in range(N_CORES):
        tgt_b = tgt[b].reshape(HW)
        c_list = np.unique(tgt_b)
        assert len(c_list) <= NCAND, f"more than {NCAND} distinct bins"
        j_pix = np.searchsorted(c_list, tgt_b)

        # group pixels by target position j into 16-pixel cells (-1 pads)
        cell_rows = []
        cell_js = []
        for j in range(len(c_list)):
            pix = np.flatnonzero(j_pix == j)
            ncell = -(-len(pix) // 16)
            pad = ncell * 16 - len(pix)
            if pad:
                pix = np.concatenate([pix, np.full(pad, -1, pix.dtype)])
            cell_rows.append(pix.reshape(ncell, 16))
            cell_js.append(np.full(ncell, j, np.int64))
        cells = np.concatenate(cell_rows)
        cj = np.concatenate(cell_js)
        assert len(cells) <= ncells_grid, f"{len(cells)} cells > {ncells_grid}"
        padc = ncells_grid - len(cells)
        if padc:
            cells = np.concatenate(
                [cells, np.full((padc, 16), -1, cells.dtype)])
            cj = np.concatenate([cj, np.zeros(padc, cj.dtype)])

        # cell m -> (group g = m // K, column k = m % K); slot partition
        # p = 16*g + q holds pixel cells[m, q]
        perm = cells.reshape(NGRP, K, 16).transpose(0, 2, 1).reshape(P, K)
        valid = perm >= 0
        slot = np.where(valid, perm, 0)

        xT = depth_logits[b].reshape(C, HW).T          # [HW, 81]
        x_pm = xT[slot].astype(f16)                    # [P, K, 81]

        r17 = depth_residuals[b].reshape(C, HW)[
            np.concatenate([c_list,
                            np.full(NCAND - len(c_list), c_list[0],
                                    c_list.dtype)])]    # [17, HW]
        rc_pm = np.zeros((P, RROWP), f16)
        rc_pm[:, :RROW] = r17.T[slot].astype(f16).reshape(P, RROW)

        rt_pm = np.where(valid, res_tgt[b].reshape(HW)[slot], 0).astype(f16)
        w_pm = np.where(valid, wgt[b].reshape(HW)[slot], 0).astype(f16)

        cjk = cj.reshape(NGRP, K)                      # per (group, col) j
        ck = np.arange(K)[None, :]
        ie = ck * C + c_list[cjk]                      # es gather index
        ir = (ck * NCAND + cjk).astype(np.uint16)      # rcand gather index
        ie0 = ie[:, :KH0].astype(np.uint16)            # half 0: data base 0
        ie1 = (ie[:, KH0:] - KH0 * C).astype(np.uint16)  # half 1: rebased

        in_maps.append({
            "x": np.ascontiguousarray(x_pm.reshape(P, NEL)),
            "rc": rc_pm,
            "ie0": _wrap_idx(ie0, SIDXH),
            "ie1": _wrap_idx(ie1, SIDXH),
            "ir": _wrap_idx(ir, SIDX),
            "rt": rt_pm,
            "w": w_pm,
        })
    return in_maps


def kernel(depth_logits, depth_residuals, gt_boxes2d, num_gt_per_img, gt_center_depth):
    global LAST_RESULTS
    from concourse.bass_utils import run_bass_kernel_spmd

    depth_logits = np.ascontiguousarray(np.asarray(depth_logits, f32))
    depth_residuals = np.ascontiguousarray(np.asarray(depth_residuals, f32))

    tgt, res_tgt, wgt = _host_targets(gt_boxes2d, num_gt_per_img, gt_center_depth)
    in_maps = _build_in_maps(depth_logits.reshape(B, C, HW),
                             depth_residuals.reshape(B, C, HW),
                             tgt, res_tgt, wgt)

    nc = _get_program()
    res = run_bass_kernel_spmd(nc, in_maps, list(range(N_CORES)))
    LAST_RESULTS = res

    acc = np.zeros(2, np.float64)
    for b in range(N_CORES):
        acc += np.asarray(res.results[b]["out"], np.float64).sum(axis=0)
    num_pixels = float(B * H * W)
    map_loss = f32(-ALPHA * acc[0] / num_pixels)
    res_loss = f32(ALPHA * acc[1] / num_pixels)
    return map_loss, res_loss
